# revision 1
# baseline (speedup 1.0000x reference)
"""Trainium2 Bass kernel for nn_AutoCorrProj (3x3 conv -> local autocorr -> 5x5 regressor).

Sharding: 8 cores = (batch b in 0..3) x (H-half in 0..1). Each core computes
y[b, :, 48h:48h+48, :] from a haloed x slab.

Per-core algorithm (spatial grid = 54 rows x 102 cols, flattened row-major):
  1. conv1: f = conv3x3(x) + bias, as 18 f32r matmuls (9 taps x 2 cin-chunks of 128)
     accumulating in PSUM over 512-wide spatial chunks; ACT copies PSUM->SBUF bf16
     with per-channel bias.
  2. mask: f2 = f * mask (zero outside the true 96x96 image; kills pad-bias garbage).
  3. corr products: t_d[c,p] = f2[c,p] * f2[c,p+d] for the 25 5x5 offsets, built as
     13 stacked [128,N] bf16 DVE muls using shift-paired doubled tensors
     F2=[f2;f2], G1=[f2;f2<<1], G102=[f2;f2<<102].
  4. regressor: y = sum_s RW_s^T @ t_s (13 bf16 matmuls accumulating in PSUM) + reg_b.
"""
import sys
sys.path.insert(0, "/opt/trn_rl_repo")
import numpy as np
import ml_dtypes

import concourse.bass as bass
import concourse.mybir as mybir
import concourse.tile as tile
from concourse import bacc
from concourse.bass_utils import run_bass_kernel_spmd

B, Cin, H, W = 4, 256, 96, 96
CC, K, OC, PAD = 64, 5, 32, 2
RH, RW_ = 54, 102            # per-core padded grid rows/cols
FLAT = RH * RW_              # 5508
CONV_LO, CONV_HI = 102 + 1, FLAT - 102 - 1   # conv outputs computed on [103, 5405)
CORR_LO, CORR_HI = 3 * RW_, 51 * RW_         # corr centers on [306, 5202)
NCHUNK = 512

# (du,dv) pair list for the 13 stacked product fields.
# G1 pairs: delta_b - delta_a = 1 (dv pairs); G102 pairs: delta = 102 (du pairs).
STACKS = []
for du in range(-2, 3):
    STACKS.append(((du, -2), (du, -1), 1))
    STACKS.append(((du, 0), (du, 1), 1))
STACKS.append(((-2, 2), (-1, 2), 102))
STACKS.append(((0, 2), (1, 2), 102))
SINGLE = (2, 2)   # 25th tap, K=64 matmul off G1 top half


def _flat(du, dv):
    return du * RW_ + dv


_CACHE = {}


def _build_nc():
    if "nc" in _CACHE:
        return _CACHE["nc"]
    nc = bacc.Bacc("TRN2", target_bir_lowering=False, debug=False)
    f32, f32r, bf16 = mybir.dt.float32, mybir.dt.float32r, mybir.dt.bfloat16

    x_d = nc.dram_tensor("x", [2, 128, FLAT], f32r, kind="ExternalInput").ap()
    ew_d = nc.dram_tensor("ew", [128, 18 * 64], f32r, kind="ExternalInput").ap()
    eb_d = nc.dram_tensor("eb", [64, 1], f32, kind="ExternalInput").ap()
    rw_d = nc.dram_tensor("rw", [128, 13 * 32], bf16, kind="ExternalInput").ap()
    rb_d = nc.dram_tensor("rb", [32, 1], f32, kind="ExternalInput").ap()
    mask_d = nc.dram_tensor("mask", [64, FLAT], bf16, kind="ExternalInput").ap()
    y_d = nc.dram_tensor("y", [OC, 48, 96], f32, kind="ExternalOutput").ap()

    with tile.TileContext(nc) as tc:
        with tc.tile_pool(name="big", bufs=1) as big, \
             tc.tile_pool(name="small", bufs=1) as small, \
             tc.tile_pool(name="tmul", bufs=6) as tmul, \
             tc.tile_pool(name="psf", bufs=2, space="PSUM") as psf, \
             tc.tile_pool(name="psy", bufs=2, space="PSUM") as psy:

            X0 = big.tile([128, FLAT], f32r, tag="X0")
            X1 = big.tile([128, FLAT], f32r, tag="X1")
            for q in range(4):
                c0 = q * 1377
                nc.sync.dma_start(X0[:, c0:c0 + 1377], x_d[0, :, c0:c0 + 1377])
                nc.sync.dma_start(X1[:, c0:c0 + 1377], x_d[1, :, c0:c0 + 1377])
            EW = small.tile([128, 18 * 64], f32r, tag="EW")
            nc.sync.dma_start(EW[:], ew_d)
            EB = small.tile([64, 1], f32, tag="EB")
            nc.sync.dma_start(EB[:], eb_d)
            RWt = small.tile([128, 13 * 32], bf16, tag="RW")
            nc.sync.dma_start(RWt[:], rw_d)
            RB = small.tile([32, 1], f32, tag="RB")
            nc.sync.dma_start(RB[:], rb_d)
            MK = big.tile([64, FLAT], bf16, tag="MK")
            nc.sync.dma_start(MK[:], mask_d)

            FB = big.tile([64, FLAT], bf16, tag="FB")    # f + bias (pre-mask)
            F2 = big.tile([128, FLAT], bf16, tag="F2")   # [f2; f2]
            G1 = big.tile([128, FLAT], bf16, tag="G1")   # [f2; f2 << 1]
            G102 = big.tile([128, FLAT], bf16, tag="G102")

            # ---- Phase 1: conv3x3 ----
            conv_len = CONV_HI - CONV_LO
            nck = (conv_len + NCHUNK - 1) // NCHUNK
            for k in range(nck):
                p0 = CONV_LO + k * NCHUNK
                n = min(NCHUNK, CONV_HI - p0)
                pf = psf.tile([64, NCHUNK], mybir.dt.float32, tag="pf")
                first = True
                for ci, Xc in ((0, X0), (1, X1)):
                    for du in (-1, 0, 1):
                        for dv in (-1, 0, 1):
                            t = (du + 1) * 3 + (dv + 1)
                            wcol = 64 * (ci * 9 + t)
                            sh = _flat(du, dv)
                            nc.tensor.matmul(
                                pf[:, 0:n],
                                lhsT=EW[:, wcol:wcol + 64],
                                rhs=Xc[:, p0 + sh:p0 + sh + n],
                                start=first, stop=(ci == 1 and du == 1 and dv == 1))
                            first = False
                nc.scalar.activation(FB[:, p0:p0 + n], pf[:, 0:n],
                                     mybir.ActivationFunctionType.Identity,
                                     bias=EB[:, 0:1], scale=1.0)

            # ---- Phase 2: mask + doubled/shifted tensors ----
            nc.vector.tensor_mul(F2[0:64, :], FB[:], MK[:])
            # cross-partition doubling via SBUF->SBUF DMA
            nc.sync.dma_start(F2[64:128, :], F2[0:64, :])
            nc.vector.tensor_copy(G1[0:64, :], F2[0:64, :])
            nc.sync.dma_start(G1[64:128, 0:FLAT - 1], F2[0:64, 1:FLAT])
            nc.scalar.copy(G102[0:64, :], F2[0:64, :])
            nc.sync.dma_start(G102[64:128, 0:FLAT - 102], F2[0:64, 102:FLAT])

            # ---- Phase 3: corr products + regressor matmuls ----
            corr_len = CORR_HI - CORR_LO
            nyk = (corr_len + NCHUNK - 1) // NCHUNK
            YS = big.tile([32, corr_len], mybir.dt.float32, tag="YS")
            for k in range(nyk):
                p0 = CORR_LO + k * NCHUNK
                n = min(NCHUNK, CORR_HI - p0)
                py = psy.tile([32, NCHUNK], mybir.dt.float32, tag="py")
                for s, ((dua, dva), _pb, _sig) in enumerate(STACKS):
                    da = _flat(dua, dva)
                    G = G1 if _sig == 1 else G102
                    T = tmul.tile([128, NCHUNK], bf16, tag="T")
                    nc.vector.tensor_mul(T[:, 0:n], G[:, p0 + da:p0 + da + n],
                                         F2[:, p0:p0 + n])
                    nc.tensor.matmul(py[:, 0:n], lhsT=RWt[:, 32 * s:32 * s + 32],
                                     rhs=T[:, 0:n], start=(s == 0), stop=False)
                # 25th tap: K=64 matmul off G1 top half
                da = _flat(*SINGLE)
                T2 = tmul.tile([64, NCHUNK], bf16, tag="T2")
                nc.vector.tensor_mul(T2[:, 0:n], G1[0:64, p0 + da:p0 + da + n],
                                     F2[0:64, p0:p0 + n])
                nc.tensor.matmul(py[:, 0:n], lhsT=RWt[0:64, 384:416],
                                 rhs=T2[:, 0:n], start=False, stop=True)
                nc.scalar.activation(YS[:, k * NCHUNK:k * NCHUNK + n], py[:, 0:n],
                                     mybir.ActivationFunctionType.Identity,
                                     bias=RB[:, 0:1], scale=1.0)

            ys3 = YS[:].rearrange("o (r w) -> o r w", w=RW_)
            nc.sync.dma_start(y_d, ys3[:, :, 3:99])

    nc.finalize()
    _CACHE["nc"] = nc
    return nc


def _prep_inputs(x, extract_w, extract_b, reg_w, reg_b):
    xp = np.zeros((B, Cin, H + 6, W + 6), np.float32)
    xp[:, :, 3:99, 3:99] = np.asarray(x, np.float32)

    ew = np.zeros((128, 18 * 64), np.float32)
    ewn = np.asarray(extract_w, np.float32)
    for ci in range(2):
        for du in (-1, 0, 1):
            for dv in (-1, 0, 1):
                t = (du + 1) * 3 + (dv + 1)
                ew[:, 64 * (ci * 9 + t):64 * (ci * 9 + t) + 64] = \
                    ewn[:, 128 * ci:128 * ci + 128, du + 1, dv + 1].T
    eb = np.asarray(extract_b, np.float32).reshape(64, 1)

    rw = np.zeros((128, 13 * 32), np.float32)
    rwn = np.asarray(reg_w, np.float32)
    for s, ((dua, dva), (dub, dvb), _sig) in enumerate(STACKS):
        rw[0:64, 32 * s:32 * s + 32] = rwn[:, :, dua + 2, dva + 2].T
        rw[64:128, 32 * s:32 * s + 32] = rwn[:, :, dub + 2, dvb + 2].T
    rw[0:64, 384:416] = rwn[:, :, SINGLE[0] + 2, SINGLE[1] + 2].T
    rw = rw.astype(ml_dtypes.bfloat16)
    rb = np.asarray(reg_b, np.float32).reshape(32, 1)

    masks = []
    for h in (0, 1):
        m = np.zeros((RH, RW_), np.float32)
        for i in range(RH):
            g = 48 * h + i - 3
            if 0 <= g < H:
                m[i, 3:99] = 1.0
        masks.append(np.broadcast_to(m.reshape(1, FLAT), (64, FLAT))
                     .astype(ml_dtypes.bfloat16))

    in_maps = []
    for b in range(B):
        for h in (0, 1):
            xs = xp[b, :, 48 * h:48 * h + RH, :].reshape(2, 128, FLAT)
            in_maps.append({
                "x": np.ascontiguousarray(xs),
                "ew": ew, "eb": eb, "rw": rw, "rb": rb,
                "mask": masks[h],
            })
    return in_maps


def kernel(x, extract_w, extract_b, reg_w, reg_b):
    nc = _build_nc()
    in_maps = _prep_inputs(x, extract_w, extract_b, reg_w, reg_b)
    res = run_bass_kernel_spmd(nc, in_maps, core_ids=list(range(8)), trace=False)
    y = np.zeros((B, OC, H, W), np.float32)
    for b in range(B):
        for h in (0, 1):
            y[b, :, 48 * h:48 * h + 48, :] = res.results[2 * b + h]["y"]
    return y
